# revision 2
# baseline (speedup 1.0000x reference)
"""AttentiveStatPool Trainium2 kernel.

Full inputs -> full output; shards batch B=32 across 8 NeuronCores
(4 utterances per core), runs one SPMD Bass/Tile kernel, gathers.

Math (per utterance, per channel c):
  mean/std over T -> glob = [x; mean; std] -> h = relu(W1 @ glob + b1)
  logits = W2 @ h (+ b2, which cancels in the softmax over T and is dropped)
  w = softmax_T(logits); out = [sum_t x*w, sqrt(clip(sum_t x^2*w - mean_w^2))]

Implementation notes:
  - e = exp(logits) unnormalized; S1 = sum x*e, S2 = sum x^2*e, s = sum e
    computed with fused DVE scalar_tensor_tensor accumulate ops; the
    normalization (1/s) is applied to the tiny [128, 12] results.
  - x is cast to bf16 by an ACT Copy-with-accumulate pass that also yields
    sum(x); sum(x^2) comes from ACT Square-accum / DVE STT (split to
    balance the two engines). Matmuls are bf16 (fp32 PSUM accumulate).
  - std = exp(0.5*ln(var)) so every ACT function (copy, square, relu, ln,
    exp) lives in one table set (no table-switch stalls).
"""

import numpy as np
import ml_dtypes
from contextlib import ExitStack

import concourse.bass as bass
import concourse.tile as tile
from concourse import mybir
from concourse.bass_utils import run_bass_kernel_spmd

B, C, T, BOT = 32, 1536, 2000, 128
NCORES = 8
BS = B // NCORES          # utterances per core
NCH = C // 128            # channel chunks
EPS = 1e-4
F32 = mybir.dt.float32
BF16 = mybir.dt.bfloat16
MULT = mybir.AluOpType.mult
AF = mybir.ActivationFunctionType

_counter = [0]


def _split_excess_waits(nc, cap_regular=1, cap_es=2):
    """Walrus allows 1 sem-wait per regular instruction (2 on
    EventSemaphore). Hoist excess waits onto EventSemaphore insts."""
    for f in nc.m.functions:
        for blk in f.blocks:
            insts = blk.instructions
            out = []
            for inst in insts:
                si = inst.sync_info
                cap = (
                    cap_es
                    if isinstance(inst, mybir.InstEventSemaphore)
                    else cap_regular
                )
                if si is not None and len(si.on_wait) > cap:
                    waits = list(si.on_wait)
                    keep, extra = waits[:cap], waits[cap:]
                    for i in range(0, len(extra), 2):
                        _counter[0] += 1
                        es = mybir.InstEventSemaphore(
                            name=f"waitsplit_{_counter[0]}", engine=inst.engine
                        )
                        es.sync_info = mybir.SyncInfo(
                            on_wait=extra[i : i + 2], on_update=[]
                        )
                        out.append(es)
                    inst.sync_info = mybir.SyncInfo(
                        on_wait=keep, on_update=list(si.on_update)
                    )
                out.append(inst)
            if len(out) != len(insts):
                insts.clear()
                insts.extend(out)


def _build(ctx, tc):
    nc = tc.nc
    x_in = nc.dram_tensor("x", [BS, C, T], F32, kind="ExternalInput").ap()
    w1xt_in = nc.dram_tensor("w1xt", [C, BOT], BF16, kind="ExternalInput").ap()
    w1mt_in = nc.dram_tensor("w1mt", [C, BOT], BF16, kind="ExternalInput").ap()
    w1st_in = nc.dram_tensor("w1st", [C, BOT], BF16, kind="ExternalInput").ap()
    w2t_in = nc.dram_tensor("w2t", [BOT, C], BF16, kind="ExternalInput").ap()
    b1_in = nc.dram_tensor("b1", [BOT, 1], F32, kind="ExternalInput").ap()
    out_dram = nc.dram_tensor("out", [BS, 2 * C], F32, kind="ExternalOutput").ap()

    wpool = ctx.enter_context(tc.tile_pool(name="weights", bufs=1))
    xfpool = ctx.enter_context(tc.tile_pool(name="xf", bufs=4))
    xbpool = ctx.enter_context(tc.tile_pool(name="xb", bufs=14))
    epool = ctx.enter_context(tc.tile_pool(name="e", bufs=2))
    upool = ctx.enter_context(tc.tile_pool(name="u", bufs=2))
    hpool = ctx.enter_context(tc.tile_pool(name="h", bufs=2))
    spool = ctx.enter_context(tc.tile_pool(name="stats", bufs=1))
    tpool = ctx.enter_context(tc.tile_pool(name="tmp", bufs=1))
    hpsum = ctx.enter_context(tc.tile_pool(name="hpsum", bufs=1, space="PSUM"))
    lgpsum = ctx.enter_context(tc.tile_pool(name="lgpsum", bufs=2, space="PSUM"))

    # --- weights to SBUF ---
    w1xt = wpool.tile([128, NCH * BOT], BF16, tag="w1xt")
    w1mt = wpool.tile([128, NCH * BOT], BF16, tag="w1mt")
    w1st = wpool.tile([128, NCH * BOT], BF16, tag="w1st")
    w2t = wpool.tile([BOT, C], BF16, tag="w2t")
    b1sb = wpool.tile([BOT, 1], F32, tag="b1sb")
    for j in range(NCH):
        sl = bass.ts(j, BOT)
        nc.sync.dma_start(w1xt[:, sl], w1xt_in[bass.ts(j, 128), :])
        nc.sync.dma_start(w1mt[:, sl], w1mt_in[bass.ts(j, 128), :])
        nc.sync.dma_start(w1st[:, sl], w1st_in[bass.ts(j, 128), :])
    nc.sync.dma_start(w2t[:], w2t_in[:])
    nc.sync.dma_start(b1sb[:], b1_in[:])

    # --- persistent accumulators ([128, col]) ---
    sx = spool.tile([128, BS * NCH], F32, tag="sx")        # sum x
    sxx = spool.tile([128, BS * NCH], F32, tag="sxx")      # sum x^2
    sE = spool.tile([128, 2 * BS * NCH], F32, tag="sE")    # sum e (2 halves)
    S1 = spool.tile([128, BS * NCH], F32, tag="S1")        # sum x*e
    S2 = spool.tile([128, BS * NCH], F32, tag="S2")        # sum x^2*e
    scr_act = spool.tile([128, T], BF16, tag="scr_act")    # ACT dump
    scr_dve = spool.tile([128, T], BF16, tag="scr_dve")    # DVE dump

    NTOT = BS * NCH
    # N-subtile boundaries (bank-aligned, <=512)
    NS = [(0, 512), (512, 512), (1024, 512), (1536, 464)]

    for b in range(BS):
        hps = hpsum.tile([128, T], F32, tag="hps")
        xbs = []
        for j in range(NCH):
            col = b * NCH + j
            xf = xfpool.tile([128, T], F32, tag="xf")
            nc.sync.dma_start(xf[:], x_in[b, bass.ts(j, 128), :])
            xb = xbpool.tile([128, T], BF16, tag="xb")
            xbs.append(xb)
            # cast + sum(x) on ACT
            nc.scalar.activation(
                xb[:], xf[:], AF.Copy, accum_out=sx[:, col : col + 1]
            )
            # sum(x^2): alternate ACT / DVE to balance engines
            if j % 2 == 0:
                nc.scalar.activation(
                    scr_act[:], xf[:], AF.Square,
                    accum_out=sxx[:, col : col + 1],
                )
            else:
                nc.vector.scalar_tensor_tensor(
                    scr_dve[:], xb[:], 1.0, xb[:],
                    op0=MULT, op1=MULT,
                    accum_out=sxx[:, col : col + 1],
                )
            # stage B: h += W1x_j.T-chunk @ x_j
            for (n0, nn) in NS:
                nc.tensor.matmul(
                    hps[:, n0 : n0 + nn],
                    w1xt[:, bass.ts(j, BOT)],
                    xb[:, n0 : n0 + nn],
                    start=(j == 0),
                    stop=(j == NCH - 1),
                )

        # --- stats -> mean, std (bf16 for the matvec) ---
        bsl = slice(b * NCH, (b + 1) * NCH)
        mean_b = tpool.tile([128, NCH], BF16, tag=f"mean{b}")
        std_b = tpool.tile([128, NCH], BF16, tag=f"std{b}")
        t1 = tpool.tile([128, NCH], F32, tag=f"t1_{b}")
        t2 = tpool.tile([128, NCH], F32, tag=f"t2_{b}")
        t3 = tpool.tile([128, NCH], F32, tag=f"t3_{b}")
        t4 = tpool.tile([128, NCH], F32, tag=f"t4_{b}")
        nc.vector.tensor_scalar(mean_b[:], sx[:, bsl], 1.0 / T, None, op0=MULT)
        # var = sxx/(T-1) - sx^2/(T*(T-1))
        nc.vector.tensor_scalar(t1[:], sxx[:, bsl], 1.0 / (T - 1), None, op0=MULT)
        nc.vector.scalar_tensor_tensor(
            t2[:], sx[:, bsl], -1.0 / (T * (T - 1.0)), sx[:, bsl],
            op0=MULT, op1=MULT,
        )
        nc.vector.tensor_add(t3[:], t1[:], t2[:])
        nc.vector.tensor_scalar_max(t4[:], t3[:], EPS)
        lnv = tpool.tile([128, NCH], F32, tag=f"lnv{b}")
        nc.scalar.activation(lnv[:], t4[:], AF.Ln)
        nc.scalar.activation(std_b[:], lnv[:], AF.Exp, scale=0.5)

        # --- c_b = W1m @ mean + W1s @ std  (24 N=1 matmuls) ---
        cbp = lgpsum.tile([128, 1], F32, tag="lg")
        for j in range(NCH):
            nc.tensor.matmul(
                cbp[:], w1mt[:, bass.ts(j, BOT)], mean_b[:, j : j + 1],
                start=(j == 0), stop=False,
            )
        for j in range(NCH):
            nc.tensor.matmul(
                cbp[:], w1st[:, bass.ts(j, BOT)], std_b[:, j : j + 1],
                start=False, stop=(j == NCH - 1),
            )
        cb = tpool.tile([128, 1], F32, tag=f"cb{b}")
        nc.vector.tensor_add(cb[:], cbp[:], b1sb[:])

        # --- h = relu(hpsum + c_b) -> bf16 ---
        hsb = hpool.tile([BOT, T], BF16, tag="hsb")
        nc.scalar.activation(hsb[:], hps[:], AF.Relu, bias=cb[:])

        # --- stage C per chunk ---
        for j in range(NCH):
            col = b * NCH + j
            wsl = bass.ts(j, BOT)  # chunk of w2t columns (c-block)
            lg0 = lgpsum.tile([128, 1024], F32, tag="lg")
            lg1 = lgpsum.tile([128, 1024], F32, tag="lg")
            nc.tensor.matmul(
                lg0[:, 0:512], w2t[:, wsl], hsb[:, 0:512], start=True, stop=True
            )
            nc.tensor.matmul(
                lg0[:, 512:1024], w2t[:, wsl], hsb[:, 512:1024],
                start=True, stop=True,
            )
            nc.tensor.matmul(
                lg1[:, 0:512], w2t[:, wsl], hsb[:, 1024:1536],
                start=True, stop=True,
            )
            nc.tensor.matmul(
                lg1[:, 512:976], w2t[:, wsl], hsb[:, 1536:2000],
                start=True, stop=True,
            )
            e = epool.tile([128, T], BF16, tag="e")
            nc.scalar.activation(
                e[:, 0:1024], lg0[:], AF.Exp, accum_out=sE[:, col : col + 1]
            )
            nc.scalar.activation(
                e[:, 1024:2000], lg1[:, 0:976], AF.Exp,
                accum_out=sE[:, NTOT + col : NTOT + col + 1],
            )
            u = upool.tile([128, T], BF16, tag="u")
            nc.vector.scalar_tensor_tensor(
                u[:], xbs[j][:], 1.0, e[:],
                op0=MULT, op1=MULT, accum_out=S1[:, col : col + 1],
            )
            nc.vector.scalar_tensor_tensor(
                scr_dve[:], xbs[j][:], 1.0, u[:],
                op0=MULT, op1=MULT, accum_out=S2[:, col : col + 1],
            )

    # --- finalize (batched over all b) ---
    fs = spool.tile([128, NTOT], F32, tag="fs")
    rs = spool.tile([128, NTOT], F32, tag="rs")
    wmean = spool.tile([128, NTOT], F32, tag="wmean")
    e2w = spool.tile([128, NTOT], F32, tag="e2w")
    nm2 = spool.tile([128, NTOT], F32, tag="nm2")
    varw = spool.tile([128, NTOT], F32, tag="varw")
    varc = spool.tile([128, NTOT], F32, tag="varc")
    lnw = spool.tile([128, NTOT], F32, tag="lnw")
    wsd = spool.tile([128, NTOT], F32, tag="wsd")
    nc.vector.tensor_add(fs[:], sE[:, 0:NTOT], sE[:, NTOT : 2 * NTOT])
    nc.vector.reciprocal(rs[:], fs[:])
    nc.vector.tensor_mul(wmean[:], S1[:], rs[:])
    nc.vector.tensor_mul(e2w[:], S2[:], rs[:])
    nc.vector.scalar_tensor_tensor(
        nm2[:], wmean[:], -1.0, wmean[:], op0=MULT, op1=MULT
    )
    nc.vector.tensor_add(varw[:], e2w[:], nm2[:])
    nc.vector.tensor_scalar_max(varc[:], varw[:], EPS)
    nc.scalar.activation(lnw[:], varc[:], AF.Ln)
    nc.scalar.activation(wsd[:], lnw[:], AF.Exp, scale=0.5)
    for b in range(BS):
        for j in range(NCH):
            col = b * NCH + j
            nc.sync.dma_start(
                out_dram[b, j * 128 : (j + 1) * 128], wmean[:, col : col + 1]
            )
            nc.sync.dma_start(
                out_dram[b, C + j * 128 : C + (j + 1) * 128],
                wsd[:, col : col + 1],
            )


_NC_CACHE = {}


def _get_nc():
    if "nc" not in _NC_CACHE:
        nc = bass.Bass("TRN2", target_bir_lowering=False, debug=False)
        with tile.TileContext(nc) as tc:
            with ExitStack() as ctx:
                _build(ctx, tc)
        _split_excess_waits(nc)
        _NC_CACHE["nc"] = nc
    return _NC_CACHE["nc"]


def kernel(x, W1, b1, W2, b2, _trace=False, _trace_kwargs=None):
    x = np.asarray(x, dtype=np.float32)
    W1 = np.asarray(W1, dtype=np.float32)
    b1 = np.asarray(b1, dtype=np.float32)
    W2 = np.asarray(W2, dtype=np.float32)
    b2 = np.asarray(b2, dtype=np.float32)  # cancels in softmax; unused

    bf = ml_dtypes.bfloat16
    w1xt = np.ascontiguousarray(W1[:, 0:C].T).astype(bf)          # [C, BOT]
    w1mt = np.ascontiguousarray(W1[:, C : 2 * C].T).astype(bf)    # [C, BOT]
    w1st = np.ascontiguousarray(W1[:, 2 * C : 3 * C].T).astype(bf)
    w2t = np.ascontiguousarray(W2.T).astype(bf)                   # [BOT, C]
    b1c = np.ascontiguousarray(b1.reshape(BOT, 1))

    nc = _get_nc()
    in_maps = [
        {
            "x": np.ascontiguousarray(x[i * BS : (i + 1) * BS]),
            "w1xt": w1xt,
            "w1mt": w1mt,
            "w1st": w1st,
            "w2t": w2t,
            "b1": b1c,
        }
        for i in range(NCORES)
    ]
    res = run_bass_kernel_spmd(
        nc,
        in_maps,
        list(range(NCORES)),
        trace=_trace,
        **(_trace_kwargs or {}),
    )
    out = np.concatenate([res.results[i]["out"] for i in range(NCORES)], axis=0)
    if _trace:
        kernel.last_results = res
    return out


# revision 9
# speedup vs baseline: 1.3252x; 1.3252x over previous
"""AttentiveStatPool Trainium2 kernel.

Full inputs -> full output; shards batch B=32 across 8 NeuronCores
(4 utterances per core), runs one SPMD Bass/Tile kernel, gathers.

Math (per utterance, per channel c):
  mean/std over T -> glob = [x; mean; std] -> h = relu(W1 @ glob + b1)
  logits = W2 @ h (+ b2, which cancels in the softmax over T and is dropped)
  w = softmax_T(logits); out = [sum_t x*w, sqrt(clip(sum_t x^2*w - mean_w^2))]

Implementation notes:
  - e = exp(logits) unnormalized; S1 = sum x*e, S2 = sum x^2*e, s = sum e
    computed with fused DVE scalar_tensor_tensor accumulate ops; the
    normalization (1/s) is applied to the tiny [128, 12] results.
  - x is cast to bf16 by an ACT Copy-with-accumulate pass that also yields
    sum(x); sum(x^2) comes from ACT Square-accum / DVE STT (split to
    balance the two engines). Matmuls are bf16 (fp32 PSUM accumulate).
  - std = exp(0.5*ln(var)) so every ACT function (copy, square, relu, ln,
    exp) lives in one table set (no table-switch stalls).
"""

import numpy as np
import ml_dtypes
from contextlib import ExitStack

import concourse.bass as bass
import concourse.tile as tile
from concourse import mybir
from concourse.bass_utils import run_bass_kernel_spmd

B, C, T, BOT = 32, 1536, 2000, 128
NCORES = 8
BS = B // NCORES          # utterances per core
NCH = C // 128            # channel chunks
EPS = 1e-4
F32 = mybir.dt.float32
BF16 = mybir.dt.bfloat16
MULT = mybir.AluOpType.mult
AF = mybir.ActivationFunctionType

_counter = [0]


def _split_excess_waits(nc, cap_regular=1, cap_es=2):
    """Walrus allows 1 sem-wait per regular instruction (2 on
    EventSemaphore). Hoist excess waits onto EventSemaphore insts."""
    for f in nc.m.functions:
        for blk in f.blocks:
            insts = blk.instructions
            out = []
            for inst in insts:
                si = inst.sync_info
                cap = (
                    cap_es
                    if isinstance(inst, mybir.InstEventSemaphore)
                    else cap_regular
                )
                if si is not None and len(si.on_wait) > cap:
                    waits = list(si.on_wait)
                    keep, extra = waits[:cap], waits[cap:]
                    for i in range(0, len(extra), 2):
                        _counter[0] += 1
                        es = mybir.InstEventSemaphore(
                            name=f"waitsplit_{_counter[0]}", engine=inst.engine
                        )
                        es.sync_info = mybir.SyncInfo(
                            on_wait=extra[i : i + 2], on_update=[]
                        )
                        out.append(es)
                    inst.sync_info = mybir.SyncInfo(
                        on_wait=keep, on_update=list(si.on_update)
                    )
                out.append(inst)
            if len(out) != len(insts):
                insts.clear()
                insts.extend(out)


def _build(ctx, tc):
    nc = tc.nc
    x_in = nc.dram_tensor("x", [BS, C, T], F32, kind="ExternalInput").ap()
    w1xt_in = nc.dram_tensor("w1xt", [C, BOT], BF16, kind="ExternalInput").ap()
    w1mt_in = nc.dram_tensor("w1mt", [C, BOT], BF16, kind="ExternalInput").ap()
    w1st_in = nc.dram_tensor("w1st", [C, BOT], BF16, kind="ExternalInput").ap()
    w2t_in = nc.dram_tensor("w2t", [BOT, C], BF16, kind="ExternalInput").ap()
    b1_in = nc.dram_tensor("b1", [BOT, 1], F32, kind="ExternalInput").ap()
    ident_in = nc.dram_tensor("ident", [128, 128], F32, kind="ExternalInput").ap()
    out_dram = nc.dram_tensor("out", [BS, 2 * C], F32, kind="ExternalOutput").ap()

    wpool = ctx.enter_context(tc.tile_pool(name="weights", bufs=1))
    xfpool = ctx.enter_context(tc.tile_pool(name="xf", bufs=4))
    xbpool = ctx.enter_context(tc.tile_pool(name="xb", bufs=14))
    epool = ctx.enter_context(tc.tile_pool(name="e", bufs=2))
    upool = ctx.enter_context(tc.tile_pool(name="u", bufs=2))
    hpool = ctx.enter_context(tc.tile_pool(name="h", bufs=2))
    spool = ctx.enter_context(tc.tile_pool(name="stats", bufs=1))
    tpool = ctx.enter_context(tc.tile_pool(name="tmp", bufs=1))
    hpsum = ctx.enter_context(tc.tile_pool(name="hpsum", bufs=1, space="PSUM"))
    lgpsum = ctx.enter_context(tc.tile_pool(name="lgpsum", bufs=2, space="PSUM"))

    # --- weights to SBUF ---
    w1xt = wpool.tile([128, NCH * BOT], BF16, tag="w1xt")
    w1mt = wpool.tile([128, NCH * BOT], BF16, tag="w1mt")
    w1st = wpool.tile([128, NCH * BOT], BF16, tag="w1st")
    w2t = wpool.tile([BOT, C], BF16, tag="w2t")
    b1sb = wpool.tile([BOT, 1], F32, tag="b1sb")
    ident = wpool.tile([128, 128], F32, tag="ident")
    # weight loads: single coalesced DMAs on the (otherwise idle) gpsimd queue
    for wt, win in ((w1xt, w1xt_in), (w1mt, w1mt_in), (w1st, w1st_in)):
        nc.gpsimd.dma_start(
            wt[:].rearrange("c (j o) -> c j o", o=BOT),
            win.rearrange("(j c) o -> c j o", c=128),
        )
    nc.gpsimd.dma_start(w2t[:], w2t_in[:])
    nc.gpsimd.dma_start(b1sb[:], b1_in[:])
    nc.gpsimd.dma_start(ident[:], ident_in[:])

    # --- persistent accumulators ([128, col]) ---
    sx = spool.tile([128, BS * NCH], F32, tag="sx")        # sum x
    sxx = spool.tile([128, BS * NCH], F32, tag="sxx")      # sum x^2
    sE = spool.tile([128, 2 * BS * NCH], F32, tag="sE")    # sum e (2 halves)
    S1 = spool.tile([128, BS * NCH], F32, tag="S1")        # sum x*e
    S2 = spool.tile([128, BS * NCH], F32, tag="S2")        # sum x^2*e
    scr_act = spool.tile([128, T], BF16, tag="scr_act")    # ACT dump
    scr_dve = spool.tile([128, T], BF16, tag="scr_dve")    # DVE dump

    NTOT = BS * NCH
    # N-subtile boundaries (bank-aligned, <=512)
    NS = [(0, 512), (512, 512), (1024, 512), (1536, 464)]

    for b in range(BS):
        hps = hpsum.tile([128, T], F32, tag="hps")
        xbs = []
        for j in range(NCH):
            col = b * NCH + j
            xf = xfpool.tile([128, T], F32, tag="xf")
            nc.sync.dma_start(xf[:], x_in[b, bass.ts(j, 128), :])
            xb = xbpool.tile([128, T], BF16, tag="xb")
            xbs.append(xb)
            # cast + sum(x) on ACT
            nc.scalar.activation(
                xb[:], xf[:], AF.Copy, accum_out=sx[:, col : col + 1]
            )
            # sum(x^2): alternate ACT / DVE to balance engines
            if j % 2 == 0:
                nc.scalar.activation(
                    scr_act[:], xf[:], AF.Square,
                    accum_out=sxx[:, col : col + 1],
                )
            else:
                nc.vector.scalar_tensor_tensor(
                    scr_dve[:], xb[:], 1.0, xb[:],
                    op0=MULT, op1=MULT,
                    accum_out=sxx[:, col : col + 1],
                )
            # stage B: h += W1x_j.T-chunk @ x_j
            for (n0, nn) in NS:
                nc.tensor.matmul(
                    hps[:, n0 : n0 + nn],
                    w1xt[:, bass.ts(j, BOT)],
                    xb[:, n0 : n0 + nn],
                    start=(j == 0),
                    stop=(j == NCH - 1),
                )

        # --- stats -> mean, std (bf16 for the matvec) ---
        bsl = slice(b * NCH, (b + 1) * NCH)
        mean_b = tpool.tile([128, NCH], BF16, tag=f"mean{b}")
        std_b = tpool.tile([128, NCH], BF16, tag=f"std{b}")
        t1 = tpool.tile([128, NCH], F32, tag=f"t1_{b}")
        t2 = tpool.tile([128, NCH], F32, tag=f"t2_{b}")
        t3 = tpool.tile([128, NCH], F32, tag=f"t3_{b}")
        t4 = tpool.tile([128, NCH], F32, tag=f"t4_{b}")
        nc.vector.tensor_scalar(mean_b[:], sx[:, bsl], 1.0 / T, None, op0=MULT)
        # var = sxx/(T-1) - sx^2/(T*(T-1))
        nc.vector.tensor_scalar(t1[:], sxx[:, bsl], 1.0 / (T - 1), None, op0=MULT)
        nc.vector.scalar_tensor_tensor(
            t2[:], sx[:, bsl], -1.0 / (T * (T - 1.0)), sx[:, bsl],
            op0=MULT, op1=MULT,
        )
        nc.vector.tensor_add(t3[:], t1[:], t2[:])
        nc.vector.tensor_scalar_max(t4[:], t3[:], EPS)
        lnv = tpool.tile([128, NCH], F32, tag=f"lnv{b}")
        nc.scalar.activation(lnv[:], t4[:], AF.Ln)
        nc.scalar.activation(std_b[:], lnv[:], AF.Exp, scale=0.5)

        # --- c_b = W1m @ mean + W1s @ std  (24 N=1 matmuls) ---
        cbp = lgpsum.tile([128, 1], F32, tag="lg")
        for j in range(NCH):
            nc.tensor.matmul(
                cbp[:], w1mt[:, bass.ts(j, BOT)], mean_b[:, j : j + 1],
                start=(j == 0), stop=False,
            )
        for j in range(NCH):
            nc.tensor.matmul(
                cbp[:], w1st[:, bass.ts(j, BOT)], std_b[:, j : j + 1],
                start=False, stop=(j == NCH - 1),
            )
        cb = tpool.tile([128, 1], F32, tag=f"cb{b}")
        nc.vector.tensor_add(cb[:], cbp[:], b1sb[:])

        # --- h = relu(hpsum + c_b) -> bf16 ---
        hsb = hpool.tile([BOT, T], BF16, tag="hsb")
        nc.scalar.activation(hsb[:], hps[:], AF.Relu, bias=cb[:])

        # --- stage C per chunk ---
        for j in range(NCH):
            col = b * NCH + j
            wsl = bass.ts(j, BOT)  # chunk of w2t columns (c-block)
            lg0 = lgpsum.tile([128, 1024], F32, tag="lg")
            lg1 = lgpsum.tile([128, 1024], F32, tag="lg")
            nc.tensor.matmul(
                lg0[:, 0:512], w2t[:, wsl], hsb[:, 0:512], start=True, stop=True
            )
            nc.tensor.matmul(
                lg0[:, 512:1024], w2t[:, wsl], hsb[:, 512:1024],
                start=True, stop=True,
            )
            nc.tensor.matmul(
                lg1[:, 0:512], w2t[:, wsl], hsb[:, 1024:1536],
                start=True, stop=True,
            )
            nc.tensor.matmul(
                lg1[:, 512:976], w2t[:, wsl], hsb[:, 1536:2000],
                start=True, stop=True,
            )
            e = epool.tile([128, T], BF16, tag="e")
            nc.scalar.activation(
                e[:, 0:1024], lg0[:], AF.Exp, accum_out=sE[:, col : col + 1]
            )
            nc.scalar.activation(
                e[:, 1024:2000], lg1[:, 0:976], AF.Exp,
                accum_out=sE[:, NTOT + col : NTOT + col + 1],
            )
            u = upool.tile([128, T], BF16, tag="u")
            nc.vector.scalar_tensor_tensor(
                u[:], xbs[j][:], 1.0, e[:],
                op0=MULT, op1=MULT, accum_out=S1[:, col : col + 1],
            )
            nc.vector.scalar_tensor_tensor(
                scr_dve[:], xbs[j][:], 1.0, u[:],
                op0=MULT, op1=MULT, accum_out=S2[:, col : col + 1],
            )

    # --- finalize (batched over all b) ---
    fs = spool.tile([128, NTOT], F32, tag="fs")
    rs = spool.tile([128, NTOT], F32, tag="rs")
    wmean = spool.tile([128, NTOT], F32, tag="wmean")
    e2w = spool.tile([128, NTOT], F32, tag="e2w")
    nm2 = spool.tile([128, NTOT], F32, tag="nm2")
    varw = spool.tile([128, NTOT], F32, tag="varw")
    varc = spool.tile([128, NTOT], F32, tag="varc")
    lnw = spool.tile([128, NTOT], F32, tag="lnw")
    wsd = spool.tile([128, NTOT], F32, tag="wsd")
    nc.vector.tensor_add(fs[:], sE[:, 0:NTOT], sE[:, NTOT : 2 * NTOT])
    nc.vector.reciprocal(rs[:], fs[:])
    nc.vector.tensor_mul(wmean[:], S1[:], rs[:])
    nc.vector.tensor_mul(e2w[:], S2[:], rs[:])
    nc.vector.scalar_tensor_tensor(
        nm2[:], wmean[:], -1.0, wmean[:], op0=MULT, op1=MULT
    )
    nc.vector.tensor_add(varw[:], e2w[:], nm2[:])
    nc.vector.tensor_scalar_max(varc[:], varw[:], EPS)
    nc.scalar.activation(lnw[:], varc[:], AF.Ln)
    nc.scalar.activation(wsd[:], lnw[:], AF.Exp, scale=0.5)
    # transpose [128, 48] -> [48, 128] on PE, then 2 contiguous stores
    wmT = lgpsum.tile([NTOT, 128], F32, tag="lg")
    nc.tensor.transpose(wmT[:], wmean[:], ident[:])
    wsT = lgpsum.tile([NTOT, 128], F32, tag="lg")
    nc.tensor.transpose(wsT[:], wsd[:], ident[:])
    wmTs = spool.tile([NTOT, 128], F32, tag="wmTs")
    wsTs = spool.tile([NTOT, 128], F32, tag="wsTs")
    nc.vector.tensor_copy(wmTs[:], wmT[:])
    nc.vector.tensor_copy(wsTs[:], wsT[:])
    for b in range(BS):
        nc.sync.dma_start(
            out_dram[b, 0:C].rearrange("(j p) -> j p", p=128),
            wmTs[b * NCH : (b + 1) * NCH, :],
        )
        nc.sync.dma_start(
            out_dram[b, C : 2 * C].rearrange("(j p) -> j p", p=128),
            wsTs[b * NCH : (b + 1) * NCH, :],
        )


_NC_CACHE = {}


def _get_nc():
    if "nc" not in _NC_CACHE:
        nc = bass.Bass("TRN2", target_bir_lowering=False, debug=False)
        with tile.TileContext(nc) as tc:
            with ExitStack() as ctx:
                _build(ctx, tc)
        _split_excess_waits(nc)
        _NC_CACHE["nc"] = nc
    return _NC_CACHE["nc"]


def kernel(x, W1, b1, W2, b2, _trace=False, _trace_kwargs=None):
    x = np.asarray(x, dtype=np.float32)
    W1 = np.asarray(W1, dtype=np.float32)
    b1 = np.asarray(b1, dtype=np.float32)
    W2 = np.asarray(W2, dtype=np.float32)
    b2 = np.asarray(b2, dtype=np.float32)  # cancels in softmax; unused

    bf = ml_dtypes.bfloat16
    w1xt = np.ascontiguousarray(W1[:, 0:C].T).astype(bf)          # [C, BOT]
    w1mt = np.ascontiguousarray(W1[:, C : 2 * C].T).astype(bf)    # [C, BOT]
    w1st = np.ascontiguousarray(W1[:, 2 * C : 3 * C].T).astype(bf)
    w2t = np.ascontiguousarray(W2.T).astype(bf)                   # [BOT, C]
    b1c = np.ascontiguousarray(b1.reshape(BOT, 1))
    ident = np.eye(128, dtype=np.float32)

    nc = _get_nc()
    in_maps = [
        {
            "x": np.ascontiguousarray(x[i * BS : (i + 1) * BS]),
            "w1xt": w1xt,
            "w1mt": w1mt,
            "w1st": w1st,
            "w2t": w2t,
            "b1": b1c,
            "ident": ident,
        }
        for i in range(NCORES)
    ]
    res = run_bass_kernel_spmd(
        nc,
        in_maps,
        list(range(NCORES)),
        trace=_trace,
        **(_trace_kwargs or {}),
    )
    out = np.concatenate([res.results[i]["out"] for i in range(NCORES)], axis=0)
    if _trace:
        kernel.last_results = res
    return out


# revision 12
# speedup vs baseline: 1.3695x; 1.0334x over previous
"""AttentiveStatPool Trainium2 kernel.

Full inputs -> full output; shards batch B=32 across 8 NeuronCores
(4 utterances per core), runs one SPMD Bass/Tile kernel, gathers.

Math (per utterance, per channel c):
  mean/std over T -> glob = [x; mean; std] -> h = relu(W1 @ glob + b1)
  logits = W2 @ h (+ b2, which cancels in the softmax over T and is dropped)
  w = softmax_T(logits); out = [sum_t x*w, sqrt(clip(sum_t x^2*w - mean_w^2))]

Implementation notes:
  - e = exp(logits) unnormalized; S1 = sum x*e, S2 = sum x^2*e, s = sum e
    computed with fused DVE scalar_tensor_tensor accumulate ops; the
    normalization (1/s) is applied to the tiny [128, 12] results.
  - x is cast to bf16 by an ACT Copy-with-accumulate pass that also yields
    sum(x); sum(x^2) comes from ACT Square-accum / DVE STT (split to
    balance the two engines). Matmuls are bf16 (fp32 PSUM accumulate).
  - std = exp(0.5*ln(var)) so every ACT function (copy, square, relu, ln,
    exp) lives in one table set (no table-switch stalls).
"""

import numpy as np
import ml_dtypes
from contextlib import ExitStack

import concourse.bass as bass
import concourse.tile as tile
from concourse import mybir
from concourse.bass_utils import run_bass_kernel_spmd

B, C, T, BOT = 32, 1536, 2000, 128
NCORES = 8
BS = B // NCORES          # utterances per core
NCH = C // 128            # channel chunks
EPS = 1e-4
F32 = mybir.dt.float32
BF16 = mybir.dt.bfloat16
MULT = mybir.AluOpType.mult
AF = mybir.ActivationFunctionType

_counter = [0]


def _split_excess_waits(nc, cap_regular=1, cap_es=2):
    """Walrus allows 1 sem-wait per regular instruction (2 on
    EventSemaphore). Hoist excess waits onto EventSemaphore insts."""
    for f in nc.m.functions:
        for blk in f.blocks:
            insts = blk.instructions
            out = []
            for inst in insts:
                si = inst.sync_info
                cap = (
                    cap_es
                    if isinstance(inst, mybir.InstEventSemaphore)
                    else cap_regular
                )
                if si is not None and len(si.on_wait) > cap:
                    waits = list(si.on_wait)
                    keep, extra = waits[:cap], waits[cap:]
                    for i in range(0, len(extra), 2):
                        _counter[0] += 1
                        es = mybir.InstEventSemaphore(
                            name=f"waitsplit_{_counter[0]}", engine=inst.engine
                        )
                        es.sync_info = mybir.SyncInfo(
                            on_wait=extra[i : i + 2], on_update=[]
                        )
                        out.append(es)
                    inst.sync_info = mybir.SyncInfo(
                        on_wait=keep, on_update=list(si.on_update)
                    )
                out.append(inst)
            if len(out) != len(insts):
                insts.clear()
                insts.extend(out)


def _build(ctx, tc):
    nc = tc.nc
    x_in = nc.dram_tensor("x", [BS, C, T], F32, kind="ExternalInput").ap()
    w1xt_in = nc.dram_tensor("w1xt", [C, BOT], BF16, kind="ExternalInput").ap()
    w1mt_in = nc.dram_tensor("w1mt", [C, BOT], BF16, kind="ExternalInput").ap()
    w1st_in = nc.dram_tensor("w1st", [C, BOT], BF16, kind="ExternalInput").ap()
    w2t_in = nc.dram_tensor("w2t", [BOT, C], BF16, kind="ExternalInput").ap()
    b1_in = nc.dram_tensor("b1", [BOT, 1], F32, kind="ExternalInput").ap()
    ident_in = nc.dram_tensor("ident", [128, 128], F32, kind="ExternalInput").ap()
    out_dram = nc.dram_tensor("out", [BS, 2 * C], F32, kind="ExternalOutput").ap()

    wpool = ctx.enter_context(tc.tile_pool(name="weights", bufs=1))
    xfpool = ctx.enter_context(tc.tile_pool(name="xf", bufs=4))
    xbpool = ctx.enter_context(tc.tile_pool(name="xb", bufs=24))
    epool = ctx.enter_context(tc.tile_pool(name="e", bufs=2))
    upool = ctx.enter_context(tc.tile_pool(name="u", bufs=2))
    hpool = ctx.enter_context(tc.tile_pool(name="h", bufs=2))
    spool = ctx.enter_context(tc.tile_pool(name="stats", bufs=1))
    tpool = ctx.enter_context(tc.tile_pool(name="tmp", bufs=1))
    hpsum = ctx.enter_context(tc.tile_pool(name="hpsum", bufs=1, space="PSUM"))
    lgpsum = ctx.enter_context(tc.tile_pool(name="lgpsum", bufs=1, space="PSUM"))

    # --- weights to SBUF ---
    w1xt = wpool.tile([128, NCH * BOT], BF16, tag="w1xt")
    w1mt = wpool.tile([128, NCH * BOT], BF16, tag="w1mt")
    w1st = wpool.tile([128, NCH * BOT], BF16, tag="w1st")
    w2t = wpool.tile([BOT, C], BF16, tag="w2t")
    b1sb = wpool.tile([BOT, 1], F32, tag="b1sb")
    ident = wpool.tile([128, 128], F32, tag="ident")
    # weight loads: single coalesced DMAs on the (otherwise idle) gpsimd queue
    for wt, win in ((w1xt, w1xt_in), (w1mt, w1mt_in), (w1st, w1st_in)):
        nc.gpsimd.dma_start(
            wt[:].rearrange("c (j o) -> c j o", o=BOT),
            win.rearrange("(j c) o -> c j o", c=128),
        )
    nc.gpsimd.dma_start(w2t[:], w2t_in[:])
    nc.gpsimd.dma_start(b1sb[:], b1_in[:])
    nc.gpsimd.dma_start(ident[:], ident_in[:])

    # --- persistent accumulators ([128, col]) ---
    sx = spool.tile([128, BS * NCH], F32, tag="sx")        # sum x
    sxx = spool.tile([128, BS * NCH], F32, tag="sxx")      # sum x^2
    sE = spool.tile([128, BS * NCH], F32, tag="sE")        # sum e
    S1 = spool.tile([128, BS * NCH], F32, tag="S1")        # sum x*e
    S2 = spool.tile([128, BS * NCH], F32, tag="S2")        # sum x^2*e
    scr_act = spool.tile([128, T], BF16, tag="scr_act")    # ACT dump
    scr_dve = spool.tile([128, T], BF16, tag="scr_dve")    # DVE dump

    NTOT = BS * NCH
    # N-subtile boundaries (bank-aligned, <=512)
    NS = [(0, 512), (512, 512), (1024, 512), (1536, 464)]

    hpss = {}
    hsbs = {}
    xbss = {}

    def emit_A(b):
        # h psum is [128, 2048] (exactly 4 banks); the last column doubles
        # as the c_b accumulator (disjoint from the h region [0:2000]).
        hps = hpsum.tile([128, 2048], F32, tag="hps")
        hpss[b] = hps
        xbs = []
        xbss[b] = xbs
        for j in range(NCH):
            col = b * NCH + j
            xf = xfpool.tile([128, T], F32, tag="xf")
            nc.sync.dma_start(xf[:], x_in[b, bass.ts(j, 128), :])
            xb = xbpool.tile([128, T], BF16, tag="xb")
            xbs.append(xb)
            # cast + sum(x) on ACT
            nc.scalar.activation(
                xb[:], xf[:], AF.Copy, accum_out=sx[:, col : col + 1]
            )
            # sum(x^2): alternate ACT / DVE to balance engines
            if j % 2 == 0:
                nc.scalar.activation(
                    scr_act[:], xf[:], AF.Square,
                    accum_out=sxx[:, col : col + 1],
                )
            else:
                nc.vector.scalar_tensor_tensor(
                    scr_dve[:], xb[:], 1.0, xb[:],
                    op0=MULT, op1=MULT,
                    accum_out=sxx[:, col : col + 1],
                )
            # stage B: h += W1x_j.T-chunk @ x_j
            for (n0, nn) in NS:
                nc.tensor.matmul(
                    hps[:, n0 : n0 + nn],
                    w1xt[:, bass.ts(j, BOT)],
                    xb[:, n0 : n0 + nn],
                    start=(j == 0),
                    stop=(j == NCH - 1),
                )

    def emit_B(b):
        hps = hpss[b]
        # --- stats -> mean, std (bf16 for the matvec) ---
        bsl = slice(b * NCH, (b + 1) * NCH)
        mean_b = tpool.tile([128, NCH], BF16, tag=f"mean{b}")
        std_b = tpool.tile([128, NCH], BF16, tag=f"std{b}")
        t1 = tpool.tile([128, NCH], F32, tag=f"t1_{b}")
        t2 = tpool.tile([128, NCH], F32, tag=f"t2_{b}")
        t3 = tpool.tile([128, NCH], F32, tag=f"t3_{b}")
        t4 = tpool.tile([128, NCH], F32, tag=f"t4_{b}")
        nc.vector.tensor_scalar(mean_b[:], sx[:, bsl], 1.0 / T, None, op0=MULT)
        # var = sxx/(T-1) - sx^2/(T*(T-1))
        nc.vector.tensor_scalar(t1[:], sxx[:, bsl], 1.0 / (T - 1), None, op0=MULT)
        nc.vector.scalar_tensor_tensor(
            t2[:], sx[:, bsl], -1.0 / (T * (T - 1.0)), sx[:, bsl],
            op0=MULT, op1=MULT,
        )
        nc.vector.tensor_add(t3[:], t1[:], t2[:])
        nc.vector.tensor_scalar_max(t4[:], t3[:], EPS)
        lnv = tpool.tile([128, NCH], F32, tag=f"lnv{b}")
        nc.scalar.activation(lnv[:], t4[:], AF.Ln)
        nc.scalar.activation(std_b[:], lnv[:], AF.Exp, scale=0.5)

        # --- c_b = W1m @ mean + W1s @ std  (24 N=1 matmuls) ---
        cbp = hps[:, 2047:2048]
        for j in range(NCH):
            nc.tensor.matmul(
                cbp, w1mt[:, bass.ts(j, BOT)], mean_b[:, j : j + 1],
                start=(j == 0), stop=False,
            )
        for j in range(NCH):
            nc.tensor.matmul(
                cbp, w1st[:, bass.ts(j, BOT)], std_b[:, j : j + 1],
                start=False, stop=(j == NCH - 1),
            )
        cb = tpool.tile([128, 1], F32, tag=f"cb{b}")
        nc.vector.tensor_add(cb[:], cbp, b1sb[:])

        # --- h = relu(hpsum + c_b) -> bf16 ---
        hsb = hpool.tile([BOT, T], BF16, tag="hsb")
        hsbs[b] = hsb
        nc.scalar.activation(hsb[:], hps[:, 0:T], AF.Relu, bias=cb[:])

    def emit_C(b):
        hsb = hsbs[b]
        xbs = xbss[b]
        for j in range(NCH):
            col = b * NCH + j
            wsl = bass.ts(j, BOT)  # chunk of w2t columns (c-block)
            lg = lgpsum.tile([128, 2048], F32, tag="lg")
            for (n0, nn) in NS:
                nc.tensor.matmul(
                    lg[:, n0 : n0 + nn], w2t[:, wsl], hsb[:, n0 : n0 + nn],
                    start=True, stop=True,
                )
            e = epool.tile([128, T], BF16, tag="e")
            nc.scalar.activation(
                e[:], lg[:, 0:T], AF.Exp, accum_out=sE[:, col : col + 1]
            )
            u = upool.tile([128, T], BF16, tag="u")
            nc.vector.scalar_tensor_tensor(
                u[:], xbs[j][:], 1.0, e[:],
                op0=MULT, op1=MULT, accum_out=S1[:, col : col + 1],
            )
            nc.vector.scalar_tensor_tensor(
                scr_dve[:], xbs[j][:], 1.0, u[:],
                op0=MULT, op1=MULT, accum_out=S2[:, col : col + 1],
            )

    # software-pipelined emission: B(b+1) lands right after A(b+1) so the
    # per-b serial stats/bias chain hides under C(b)'s long phase.
    emit_A(0)
    emit_B(0)
    emit_A(1)
    emit_B(1)
    emit_C(0)
    emit_A(2)
    emit_B(2)
    emit_C(1)
    emit_A(3)
    emit_B(3)
    emit_C(2)
    emit_C(3)

    # --- finalize (batched over all b) ---
    rs = spool.tile([128, NTOT], F32, tag="rs")
    wmean = spool.tile([128, NTOT], F32, tag="wmean")
    e2w = spool.tile([128, NTOT], F32, tag="e2w")
    nm2 = spool.tile([128, NTOT], F32, tag="nm2")
    varw = spool.tile([128, NTOT], F32, tag="varw")
    varc = spool.tile([128, NTOT], F32, tag="varc")
    lnw = spool.tile([128, NTOT], F32, tag="lnw")
    wsd = spool.tile([128, NTOT], F32, tag="wsd")
    nc.vector.reciprocal(rs[:], sE[:])
    nc.vector.tensor_mul(wmean[:], S1[:], rs[:])
    nc.vector.tensor_mul(e2w[:], S2[:], rs[:])
    nc.vector.scalar_tensor_tensor(
        nm2[:], wmean[:], -1.0, wmean[:], op0=MULT, op1=MULT
    )
    nc.vector.tensor_add(varw[:], e2w[:], nm2[:])
    nc.vector.tensor_scalar_max(varc[:], varw[:], EPS)
    nc.scalar.activation(lnw[:], varc[:], AF.Ln)
    nc.scalar.activation(wsd[:], lnw[:], AF.Exp, scale=0.5)
    # transpose [128, 48] -> [48, 128] on PE, then 2 contiguous stores
    wmT = lgpsum.tile([NTOT, 128], F32, tag="lg")
    nc.tensor.transpose(wmT[:], wmean[:], ident[:])
    wsT = lgpsum.tile([NTOT, 128], F32, tag="lg")
    nc.tensor.transpose(wsT[:], wsd[:], ident[:])
    wmTs = spool.tile([NTOT, 128], F32, tag="wmTs")
    wsTs = spool.tile([NTOT, 128], F32, tag="wsTs")
    nc.vector.tensor_copy(wmTs[:], wmT[:])
    nc.vector.tensor_copy(wsTs[:], wsT[:])
    for b in range(BS):
        nc.sync.dma_start(
            out_dram[b, 0:C].rearrange("(j p) -> j p", p=128),
            wmTs[b * NCH : (b + 1) * NCH, :],
        )
        nc.sync.dma_start(
            out_dram[b, C : 2 * C].rearrange("(j p) -> j p", p=128),
            wsTs[b * NCH : (b + 1) * NCH, :],
        )


_NC_CACHE = {}


def _get_nc():
    if "nc" not in _NC_CACHE:
        nc = bass.Bass("TRN2", target_bir_lowering=False, debug=False)
        with tile.TileContext(nc) as tc:
            with ExitStack() as ctx:
                _build(ctx, tc)
        _split_excess_waits(nc)
        _NC_CACHE["nc"] = nc
    return _NC_CACHE["nc"]


def kernel(x, W1, b1, W2, b2, _trace=False, _trace_kwargs=None):
    x = np.asarray(x, dtype=np.float32)
    W1 = np.asarray(W1, dtype=np.float32)
    b1 = np.asarray(b1, dtype=np.float32)
    W2 = np.asarray(W2, dtype=np.float32)
    b2 = np.asarray(b2, dtype=np.float32)  # cancels in softmax; unused

    bf = ml_dtypes.bfloat16
    w1xt = np.ascontiguousarray(W1[:, 0:C].T).astype(bf)          # [C, BOT]
    w1mt = np.ascontiguousarray(W1[:, C : 2 * C].T).astype(bf)    # [C, BOT]
    w1st = np.ascontiguousarray(W1[:, 2 * C : 3 * C].T).astype(bf)
    w2t = np.ascontiguousarray(W2.T).astype(bf)                   # [BOT, C]
    b1c = np.ascontiguousarray(b1.reshape(BOT, 1))
    ident = np.eye(128, dtype=np.float32)

    nc = _get_nc()
    in_maps = [
        {
            "x": np.ascontiguousarray(x[i * BS : (i + 1) * BS]),
            "w1xt": w1xt,
            "w1mt": w1mt,
            "w1st": w1st,
            "w2t": w2t,
            "b1": b1c,
            "ident": ident,
        }
        for i in range(NCORES)
    ]
    res = run_bass_kernel_spmd(
        nc,
        in_maps,
        list(range(NCORES)),
        trace=_trace,
        **(_trace_kwargs or {}),
    )
    out = np.concatenate([res.results[i]["out"] for i in range(NCORES)], axis=0)
    if _trace:
        kernel.last_results = res
    return out


# revision 15
# speedup vs baseline: 1.3756x; 1.0045x over previous
"""AttentiveStatPool Trainium2 kernel.

Full inputs -> full output; shards batch B=32 across 8 NeuronCores
(4 utterances per core), runs one SPMD Bass/Tile kernel, gathers.

Math (per utterance, per channel c):
  mean/std over T -> glob = [x; mean; std] -> h = relu(W1 @ glob + b1)
  logits = W2 @ h (+ b2, which cancels in the softmax over T and is dropped)
  w = softmax_T(logits); out = [sum_t x*w, sqrt(clip(sum_t x^2*w - mean_w^2))]

Implementation notes:
  - e = exp(logits) unnormalized; S1 = sum x*e, S2 = sum x^2*e, s = sum e
    computed with fused DVE scalar_tensor_tensor accumulate ops; the
    normalization (1/s) is applied to the tiny [128, 12] results.
  - x is cast to bf16 by an ACT Copy-with-accumulate pass that also yields
    sum(x); sum(x^2) comes from ACT Square-accum / DVE STT (split to
    balance the two engines). Matmuls are bf16 (fp32 PSUM accumulate).
  - std = exp(0.5*ln(var)) so every ACT function (copy, square, relu, ln,
    exp) lives in one table set (no table-switch stalls).
"""

import numpy as np
import ml_dtypes
from contextlib import ExitStack

import concourse.bass as bass
import concourse.tile as tile
from concourse import mybir
from concourse.bass_utils import run_bass_kernel_spmd

B, C, T, BOT = 32, 1536, 2000, 128
NCORES = 8
BS = B // NCORES          # utterances per core
NCH = C // 128            # channel chunks
EPS = 1e-4
F32 = mybir.dt.float32
BF16 = mybir.dt.bfloat16
MULT = mybir.AluOpType.mult
AF = mybir.ActivationFunctionType

_counter = [0]


def _split_excess_waits(nc, cap_regular=1, cap_es=2):
    """Walrus allows 1 sem-wait per regular instruction (2 on
    EventSemaphore). Hoist excess waits onto EventSemaphore insts."""
    for f in nc.m.functions:
        for blk in f.blocks:
            insts = blk.instructions
            out = []
            for inst in insts:
                si = inst.sync_info
                cap = (
                    cap_es
                    if isinstance(inst, mybir.InstEventSemaphore)
                    else cap_regular
                )
                if si is not None and len(si.on_wait) > cap:
                    waits = list(si.on_wait)
                    keep, extra = waits[:cap], waits[cap:]
                    for i in range(0, len(extra), 2):
                        _counter[0] += 1
                        es = mybir.InstEventSemaphore(
                            name=f"waitsplit_{_counter[0]}", engine=inst.engine
                        )
                        es.sync_info = mybir.SyncInfo(
                            on_wait=extra[i : i + 2], on_update=[]
                        )
                        out.append(es)
                    inst.sync_info = mybir.SyncInfo(
                        on_wait=keep, on_update=list(si.on_update)
                    )
                out.append(inst)
            if len(out) != len(insts):
                insts.clear()
                insts.extend(out)


def _build(ctx, tc):
    nc = tc.nc
    x_in = nc.dram_tensor("x", [BS, C, T], F32, kind="ExternalInput").ap()
    w1xt_in = nc.dram_tensor("w1xt", [C, BOT], BF16, kind="ExternalInput").ap()
    w1mt_in = nc.dram_tensor("w1mt", [C, BOT], BF16, kind="ExternalInput").ap()
    w1st_in = nc.dram_tensor("w1st", [C, BOT], BF16, kind="ExternalInput").ap()
    w2t_in = nc.dram_tensor("w2t", [BOT, C], BF16, kind="ExternalInput").ap()
    b1_in = nc.dram_tensor("b1", [BOT, 1], F32, kind="ExternalInput").ap()
    ident_in = nc.dram_tensor("ident", [128, 128], F32, kind="ExternalInput").ap()
    out_dram = nc.dram_tensor("out", [BS, 2 * C], F32, kind="ExternalOutput").ap()

    wpool = ctx.enter_context(tc.tile_pool(name="weights", bufs=1))
    xfpool = ctx.enter_context(tc.tile_pool(name="xf", bufs=4))
    xbpool = ctx.enter_context(tc.tile_pool(name="xb", bufs=24))
    epool = ctx.enter_context(tc.tile_pool(name="e", bufs=2))
    upool = ctx.enter_context(tc.tile_pool(name="u", bufs=2))
    hpool = ctx.enter_context(tc.tile_pool(name="h", bufs=2))
    spool = ctx.enter_context(tc.tile_pool(name="stats", bufs=1))
    tpool = ctx.enter_context(tc.tile_pool(name="tmp", bufs=1))
    hpsum = ctx.enter_context(tc.tile_pool(name="hpsum", bufs=1, space="PSUM"))
    lgpsum = ctx.enter_context(tc.tile_pool(name="lgpsum", bufs=1, space="PSUM"))

    # --- weights to SBUF ---
    w1xt = wpool.tile([128, NCH * BOT], BF16, tag="w1xt")
    w1mt = wpool.tile([128, NCH * BOT], BF16, tag="w1mt")
    w1st = wpool.tile([128, NCH * BOT], BF16, tag="w1st")
    w2t = wpool.tile([BOT, C], BF16, tag="w2t")
    b1sb = wpool.tile([BOT, 1], F32, tag="b1sb")
    ident = wpool.tile([128, 128], F32, tag="ident")
    # weight loads: single coalesced DMAs on the (otherwise idle) gpsimd queue
    for wt, win in ((w1xt, w1xt_in), (w1mt, w1mt_in), (w1st, w1st_in)):
        nc.gpsimd.dma_start(
            wt[:].rearrange("c (j o) -> c j o", o=BOT),
            win.rearrange("(j c) o -> c j o", c=128),
        )
    nc.gpsimd.dma_start(w2t[:], w2t_in[:])
    nc.gpsimd.dma_start(b1sb[:], b1_in[:])
    nc.gpsimd.dma_start(ident[:], ident_in[:])

    # --- persistent accumulators ([128, col]) ---
    sx = spool.tile([128, BS * NCH], F32, tag="sx")        # sum x
    sxx = spool.tile([128, BS * NCH], F32, tag="sxx")      # sum x^2
    sE = spool.tile([128, BS * NCH], F32, tag="sE")        # sum e
    S1 = spool.tile([128, BS * NCH], F32, tag="S1")        # sum x*e
    S2 = spool.tile([128, BS * NCH], F32, tag="S2")        # sum x^2*e
    scr_act = spool.tile([128, T], BF16, tag="scr_act")    # ACT dump
    scr_dve = spool.tile([128, T], BF16, tag="scr_dve")    # DVE dump

    NTOT = BS * NCH
    # N-subtile boundaries (bank-aligned, <=512)
    NS = [(0, 512), (512, 512), (1024, 512), (1536, 464)]

    hpss = {}
    hsbs = {}
    xbss = {}
    cbs = {}

    def emit_A(b):
        # h psum is [128, 2048] (exactly 4 banks); the last column doubles
        # as the c_b accumulator (disjoint from the h region [0:2000]).
        hps = hpsum.tile([128, 2048], F32, tag="hps")
        hpss[b] = hps
        xbs = []
        xbss[b] = xbs
        for j in range(NCH):
            col = b * NCH + j
            xf = xfpool.tile([128, T], F32, tag="xf")
            nc.sync.dma_start(xf[:], x_in[b, bass.ts(j, 128), :])
            xb = xbpool.tile([128, T], BF16, tag="xb")
            xbs.append(xb)
            # cast + sum(x) on ACT
            nc.scalar.activation(
                xb[:], xf[:], AF.Copy, accum_out=sx[:, col : col + 1]
            )
            # sum(x^2): alternate ACT / DVE to balance engines
            if j % 2 == 0:
                nc.scalar.activation(
                    scr_act[:], xf[:], AF.Square,
                    accum_out=sxx[:, col : col + 1],
                )
            else:
                nc.vector.scalar_tensor_tensor(
                    scr_dve[:], xb[:], 1.0, xb[:],
                    op0=MULT, op1=MULT,
                    accum_out=sxx[:, col : col + 1],
                )
            # stage B: h += W1x_j.T-chunk @ x_j
            for (n0, nn) in NS:
                nc.tensor.matmul(
                    hps[:, n0 : n0 + nn],
                    w1xt[:, bass.ts(j, BOT)],
                    xb[:, n0 : n0 + nn],
                    start=(j == 0),
                    stop=(j == NCH - 1),
                )

    def emit_B(b):
        hps = hpss[b]
        # --- stats -> mean, std (bf16 for the matvec) ---
        bsl = slice(b * NCH, (b + 1) * NCH)
        mean_b = tpool.tile([128, NCH], BF16, tag=f"mean{b}")
        std_b = tpool.tile([128, NCH], BF16, tag=f"std{b}")
        t1 = tpool.tile([128, NCH], F32, tag=f"t1_{b}")
        t2 = tpool.tile([128, NCH], F32, tag=f"t2_{b}")
        t3 = tpool.tile([128, NCH], F32, tag=f"t3_{b}")
        t4 = tpool.tile([128, NCH], F32, tag=f"t4_{b}")
        nc.vector.tensor_scalar(mean_b[:], sx[:, bsl], 1.0 / T, None, op0=MULT)
        # var = sxx/(T-1) - sx^2/(T*(T-1))
        nc.vector.tensor_scalar(t1[:], sxx[:, bsl], 1.0 / (T - 1), None, op0=MULT)
        nc.vector.scalar_tensor_tensor(
            t2[:], sx[:, bsl], -1.0 / (T * (T - 1.0)), sx[:, bsl],
            op0=MULT, op1=MULT,
        )
        nc.vector.tensor_add(t3[:], t1[:], t2[:])
        nc.vector.tensor_scalar_max(t4[:], t3[:], EPS)
        lnv = tpool.tile([128, NCH], F32, tag=f"lnv{b}")
        nc.scalar.activation(lnv[:], t4[:], AF.Ln)
        nc.scalar.activation(std_b[:], lnv[:], AF.Exp, scale=0.5)

        # --- c_b = W1m @ mean + W1s @ std  (24 N=1 matmuls) ---
        cbp = hps[:, 2047:2048]
        for j in range(NCH):
            nc.tensor.matmul(
                cbp, w1mt[:, bass.ts(j, BOT)], mean_b[:, j : j + 1],
                start=(j == 0), stop=False,
            )
        for j in range(NCH):
            nc.tensor.matmul(
                cbp, w1st[:, bass.ts(j, BOT)], std_b[:, j : j + 1],
                start=False, stop=(j == NCH - 1),
            )
        cb = tpool.tile([128, 1], F32, tag=f"cb{b}")
        nc.vector.tensor_add(cb[:], cbp, b1sb[:])
        cbs[b] = cb

    def emit_B2(b):
        # --- h = relu(hpsum + c_b) -> bf16 ---
        hsb = hpool.tile([BOT, T], BF16, tag="hsb")
        hsbs[b] = hsb
        nc.scalar.activation(hsb[:], hpss[b][:, 0:T], AF.Relu, bias=cbs[b][:])

    def emit_C(b, next_b=None):
        hsb = hsbs[b]
        xbs = xbss[b]
        for j in range(NCH):
            if j == 3 and next_b is not None:
                emit_B2(next_b)
            col = b * NCH + j
            wsl = bass.ts(j, BOT)  # chunk of w2t columns (c-block)
            lg = lgpsum.tile([128, 2048], F32, tag="lg")
            for (n0, nn) in NS:
                nc.tensor.matmul(
                    lg[:, n0 : n0 + nn], w2t[:, wsl], hsb[:, n0 : n0 + nn],
                    start=True, stop=True,
                )
            e = epool.tile([128, T], BF16, tag="e")
            nc.scalar.activation(
                e[:], lg[:, 0:T], AF.Exp, accum_out=sE[:, col : col + 1]
            )
            u = upool.tile([128, T], BF16, tag="u")
            nc.vector.scalar_tensor_tensor(
                u[:], xbs[j][:], 1.0, e[:],
                op0=MULT, op1=MULT, accum_out=S1[:, col : col + 1],
            )
            nc.vector.scalar_tensor_tensor(
                scr_dve[:], xbs[j][:], 1.0, u[:],
                op0=MULT, op1=MULT, accum_out=S2[:, col : col + 1],
            )

    # software-pipelined emission: B(b+1) lands right after A(b+1) so the
    # per-b serial stats/bias chain hides under C(b)'s long phase.
    emit_A(0)
    emit_B(0)
    emit_B2(0)
    emit_A(1)
    emit_B(1)
    emit_C(0, next_b=1)
    emit_A(2)
    emit_B(2)
    emit_C(1, next_b=2)
    emit_A(3)
    emit_B(3)
    emit_C(2, next_b=3)
    emit_C(3)

    # --- finalize (batched over all b) ---
    rs = spool.tile([128, NTOT], F32, tag="rs")
    wmean = spool.tile([128, NTOT], F32, tag="wmean")
    e2w = spool.tile([128, NTOT], F32, tag="e2w")
    nm2 = spool.tile([128, NTOT], F32, tag="nm2")
    varw = spool.tile([128, NTOT], F32, tag="varw")
    varc = spool.tile([128, NTOT], F32, tag="varc")
    lnw = spool.tile([128, NTOT], F32, tag="lnw")
    wsd = spool.tile([128, NTOT], F32, tag="wsd")
    nc.vector.reciprocal(rs[:], sE[:])
    nc.vector.tensor_mul(wmean[:], S1[:], rs[:])
    nc.vector.tensor_mul(e2w[:], S2[:], rs[:])
    nc.vector.scalar_tensor_tensor(
        nm2[:], wmean[:], -1.0, wmean[:], op0=MULT, op1=MULT
    )
    nc.vector.tensor_add(varw[:], e2w[:], nm2[:])
    nc.vector.tensor_scalar_max(varc[:], varw[:], EPS)
    nc.scalar.activation(lnw[:], varc[:], AF.Ln)
    nc.scalar.activation(wsd[:], lnw[:], AF.Exp, scale=0.5)
    # transpose [128, 48] -> [48, 128] on PE, then 2 contiguous stores
    wmT = lgpsum.tile([NTOT, 128], F32, tag="lg")
    nc.tensor.transpose(wmT[:], wmean[:], ident[:])
    wsT = lgpsum.tile([NTOT, 128], F32, tag="lg")
    nc.tensor.transpose(wsT[:], wsd[:], ident[:])
    wmTs = spool.tile([NTOT, 128], F32, tag="wmTs")
    wsTs = spool.tile([NTOT, 128], F32, tag="wsTs")
    nc.vector.tensor_copy(wmTs[:], wmT[:])
    nc.vector.tensor_copy(wsTs[:], wsT[:])
    for b in range(BS):
        nc.sync.dma_start(
            out_dram[b, 0:C].rearrange("(j p) -> j p", p=128),
            wmTs[b * NCH : (b + 1) * NCH, :],
        )
        nc.sync.dma_start(
            out_dram[b, C : 2 * C].rearrange("(j p) -> j p", p=128),
            wsTs[b * NCH : (b + 1) * NCH, :],
        )


_NC_CACHE = {}


def _get_nc():
    if "nc" not in _NC_CACHE:
        nc = bass.Bass("TRN2", target_bir_lowering=False, debug=False)
        with tile.TileContext(nc) as tc:
            with ExitStack() as ctx:
                _build(ctx, tc)
        _split_excess_waits(nc)
        _NC_CACHE["nc"] = nc
    return _NC_CACHE["nc"]


def kernel(x, W1, b1, W2, b2, _trace=False, _trace_kwargs=None):
    x = np.asarray(x, dtype=np.float32)
    W1 = np.asarray(W1, dtype=np.float32)
    b1 = np.asarray(b1, dtype=np.float32)
    W2 = np.asarray(W2, dtype=np.float32)
    b2 = np.asarray(b2, dtype=np.float32)  # cancels in softmax; unused

    bf = ml_dtypes.bfloat16
    w1xt = np.ascontiguousarray(W1[:, 0:C].T).astype(bf)          # [C, BOT]
    w1mt = np.ascontiguousarray(W1[:, C : 2 * C].T).astype(bf)    # [C, BOT]
    w1st = np.ascontiguousarray(W1[:, 2 * C : 3 * C].T).astype(bf)
    w2t = np.ascontiguousarray(W2.T).astype(bf)                   # [BOT, C]
    b1c = np.ascontiguousarray(b1.reshape(BOT, 1))
    ident = np.eye(128, dtype=np.float32)

    nc = _get_nc()
    in_maps = [
        {
            "x": np.ascontiguousarray(x[i * BS : (i + 1) * BS]),
            "w1xt": w1xt,
            "w1mt": w1mt,
            "w1st": w1st,
            "w2t": w2t,
            "b1": b1c,
            "ident": ident,
        }
        for i in range(NCORES)
    ]
    res = run_bass_kernel_spmd(
        nc,
        in_maps,
        list(range(NCORES)),
        trace=_trace,
        **(_trace_kwargs or {}),
    )
    out = np.concatenate([res.results[i]["out"] for i in range(NCORES)], axis=0)
    if _trace:
        kernel.last_results = res
    return out
